# revision 9
# baseline (speedup 1.0000x reference)
"""Trainium2 Bass kernel for nn_MatSurfGcn (GCN message passing, memory-bound).

Strategy (column-parallel over W_g1's output dim, 8 cores):
  reference =  enc -> gcn_conv(W_g1) -> gcn_conv(W_g2) -> head
  Both convs are linear and A @ (X @ W) == (A @ X) @ W, so the graph
  aggregation commutes out of the device entirely.  Per core c:
    x0T = relu(Wenc.T @ S)            [4096, 14]   (32 tiny PE matmuls,
                                                    written transposed)
    z_c = x0T.T @ Wv_c                [14, 1024]   Wv_c = W_g1[:,c] * w2_c
    t_c = row_sum(z_c)                [14, 1]
    host: y = head(A(A Su + b1.W_g2) + b_g2)       (two 14x14 matvecs)

  W_g2 is folded into W_g1's columns on the host (same device FLOPs, kills
  the tail multiply), and the result streams as plain bf16: 8.4 MB/core,
  so DMA (~23.4 us at 358 GB/s) and the PE bf16 column stream (~14 us)
  are both near the memory roofline.  End-to-end error ~3e-3 vs the 2e-2
  gate (bf16 quantization of x and W does not average down through the
  random-sign contraction, but starts 6x under the gate).

  The 14-node activations x0T are computed on the host and replicated to
  all cores (per the sharding hint): on the PE the encoder decomposes
  into 32 latency-bound 14-column matmuls (~13 us) that would gate the
  first z matmul and starve the W stream, while as data it is a 114 KiB
  bf16 DMA that rides along with the first W tile.
"""

import os

import numpy as np

D1, D2 = 4096, 8192
N = 14
NCORES = 8
SH = D2 // NCORES        # 1024 W_g1 columns per core
KC = D1 // 128           # 32 contraction chunks of 128
CPT = 2                  # k-chunks per full DMA tile (512 KiB bf16)
# 1-chunk tiles at both ends: earlier first matmul, shorter last-tile chain
TILE_CHUNKS = [1, 1] + [2] * 14 + [1, 1]
KSPLIT = 30              # z accumulation groups: k<30 and k>=30 (reduce overlap)
WBUFS = int(os.environ.get("KERNEL_WBUFS", "6"))
ENC_K = 18               # 6+1 mats, 3+1 cyls, 4+1 planes, 1+1 power rows

_CACHE = {}


def _build_nc():
    import concourse.bacc as bacc
    import concourse.bass as bass
    import concourse.mybir as mybir
    import concourse.tile as tile

    f32 = mybir.dt.float32
    bf16 = mybir.dt.bfloat16
    psum = bass.MemorySpace.PSUM
    alu = mybir.AluOpType

    nc = bacc.Bacc(
        "TRN2", target_bir_lowering=False, debug=False, enable_asserts=False
    )

    # host-computed x0T = relu(enc(inputs)).T, swizzled so chunk k sits at
    # cols 14k..14k+14: xb[p, k*N + n] = x0T[k*128 + p, n]
    xb_d = nc.dram_tensor("xb", [128, KC * N], bf16, kind="ExternalInput")
    # host-swizzled bf16 W_g1 shard with w2 folded in:
    # wv[p, k*SH + j] = (W_g1[k*128+p, c*SH+j] * w2[c*SH+j]) as bf16
    wv_d = nc.dram_tensor("wv", [128, KC * SH], bf16, kind="ExternalInput")
    t_d = nc.dram_tensor("t", [N, 1], f32, kind="ExternalOutput")

    with tile.TileContext(nc) as tc:
        with (
            tc.tile_pool(name="const", bufs=1) as cpool,
            tc.tile_pool(name="wvp", bufs=WBUFS) as wpool,
            tc.tile_pool(name="zps", bufs=1, space=psum) as zps,
        ):
            xb = cpool.tile([128, KC * N], bf16)
            t_sb = cpool.tile([N, 1], f32)
            ta = cpool.tile([N, 1], f32)
            tb = cpool.tile([N, 1], f32)
            tc_ = cpool.tile([N, 1], f32)
            td = cpool.tile([N, 1], f32)
            tab = cpool.tile([N, 1], f32)
            red_sc = cpool.tile([N, 512], f32)
            z_ps = zps.tile([N, SH], f32)
            z2_ps = zps.tile([N, SH], f32)

            # xb first (it gates the first matmul), then the W stream on the
            # sync HWDGE ring keeps the DMA engines saturated throughout.
            nc.sync.dma_start(out=xb[:], in_=xb_d[:])
            tiles = []  # (wt, k0, nchunks)
            k0 = 0
            for nch in TILE_CHUNKS:
                wt = wpool.tile([128, nch * SH], bf16, tag=f"wt{nch}")
                nc.sync.dma_start(
                    out=wt[:], in_=wv_d[:, k0 * SH : (k0 + nch) * SH]
                )
                tiles.append((wt, k0, nch))
                k0 += nch

            # warm the ACT engine's function table off the critical path so
            # the tail reduce doesn't pay the lazy ACT_TABLE_LOAD
            nc.scalar.activation(
                red_sc[0:1, 0:1],
                xb[0:1, 0:1],
                mybir.ActivationFunctionType.Copy,
                accum_out=td[0:1, 0:1],
            )

            # z += x0T_k.T @ Wv_k, accumulated in PSUM; two accumulation
            # groups over k so the first group's reduce overlaps the last
            # chunks' matmuls
            for wt, k0, nch in tiles:
                for a in range(nch):
                    k = k0 + a
                    zp = z_ps if k < KSPLIT else z2_ps
                    for nt in range(2):
                        nc.tensor.matmul(
                            zp[:, nt * 512 : (nt + 1) * 512],
                            xb[:, k * N : (k + 1) * N],
                            wt[:, a * SH + nt * 512 : a * SH + (nt + 1) * 512],
                            start=(k in (0, KSPLIT)),
                            stop=(k in (KSPLIT - 1, KC - 1)),
                        )

            # t = row_sum(z + z2) (w2 already folded into the W stream):
            # group-A halves reduce on DVE + ACT while group B finishes on
            # the PE; only group B's short reduce sits in the tail
            nc.vector.tensor_reduce(
                ta[:], z_ps[:, 0:512], axis=mybir.AxisListType.X, op=alu.add
            )
            nc.scalar.activation(
                red_sc[:],
                z_ps[:, 512:1024],
                mybir.ActivationFunctionType.Copy,
                accum_out=tb[:],
            )
            nc.vector.tensor_add(tab[:], ta[:], tb[:])
            nc.vector.tensor_reduce(
                tc_[:], z2_ps[:, 0:512], axis=mybir.AxisListType.X, op=alu.add
            )
            nc.scalar.activation(
                red_sc[:],
                z2_ps[:, 512:1024],
                mybir.ActivationFunctionType.Copy,
                accum_out=td[:],
            )
            nc.vector.tensor_add(tc_[:], tc_[:], td[:])
            nc.vector.tensor_add(t_sb[:], tab[:], tc_[:])
            nc.sync.dma_start(out=t_d[:], in_=t_sb[:])

    nc.compile()
    return nc


def get_nc():
    if "nc" not in _CACHE:
        _CACHE["nc"] = _build_nc()
    return _CACHE["nc"]


def build_graph_matrix(edge_index):
    """Dense normalized adjacency of the PyG-style GCNConv (self-loops +
    symmetric deg^{-1/2}); multi-edges accumulate like segment_sum does."""
    ei = np.concatenate(
        [edge_index.astype(np.int64), np.stack([np.arange(N), np.arange(N)])],
        axis=1,
    )
    src, dst = ei[0], ei[1]
    deg = np.zeros(N, np.float32)
    np.add.at(deg, dst, np.ones(len(dst), np.float32))
    dis = np.where(deg > 0, 1.0 / np.sqrt(np.maximum(deg, 1e-12)), 0.0).astype(
        np.float32
    )
    A = np.zeros((N, N), np.float32)
    np.add.at(A, (dst, src), dis[src] * dis[dst])
    return A


def build_host_inputs(inputs):
    """Per-core input maps + the graph matrix for the host epilogue."""
    f32 = np.float32
    import ml_dtypes

    bf16 = ml_dtypes.bfloat16
    mats = np.asarray(inputs["mats"], f32)
    cyls = np.asarray(inputs["cyls"], f32)
    planes = np.asarray(inputs["planes"], f32)
    power = np.asarray(inputs["power"], f32)
    edge_index = np.asarray(inputs["edge_index"])

    A = build_graph_matrix(edge_index)

    # Block-diagonal node features with bias rows of ones: x0 = relu(S.T @ Wenc)
    S = np.zeros((ENC_K, N), f32)
    S[0:6, 0:6] = mats.T
    S[6, 0:6] = 1.0
    S[7:10, 6:10] = cyls.T
    S[10, 6:10] = 1.0
    S[11:15, 10:13] = planes.T
    S[15, 10:13] = 1.0
    S[16, 13] = power[0] / 10000.0
    S[17, 13] = 1.0

    Wenc = np.ascontiguousarray(
        np.concatenate(
            [
                np.asarray(inputs["W_mat"], f32),
                np.asarray(inputs["b_mat"], f32)[None, :],
                np.asarray(inputs["W_cyl"], f32),
                np.asarray(inputs["b_cyl"], f32)[None, :],
                np.asarray(inputs["W_pl"], f32),
                np.asarray(inputs["b_pl"], f32)[None, :],
                np.asarray(inputs["W_pw"], f32),
                np.asarray(inputs["b_pw"], f32)[None, :],
            ],
            axis=0,
        )
    )
    assert Wenc.shape == (ENC_K, D1)

    W_g1 = np.asarray(inputs["W_g1"], f32)
    W_g2 = np.asarray(inputs["W_g2"], f32)

    # replicated 14-node activations, transposed + chunk-swizzled for the PE
    x0 = np.maximum(S.T @ Wenc, 0.0)  # [N, D1]
    xb = np.ascontiguousarray(
        x0.T.reshape(KC, 128, N).transpose(1, 0, 2).reshape(128, KC * N)
    ).astype(bf16)

    in_maps = []
    for c in range(NCORES):
        sl = slice(c * SH, (c + 1) * SH)
        wv = (W_g1[:, sl] * W_g2[sl, 0][None, :]).astype(bf16)  # [D1, SH]
        # swizzle so chunk k sits at cols k*SH..(k+1)*SH of a [128, *] image
        wsw = np.ascontiguousarray(
            wv.reshape(KC, 128, SH).transpose(1, 0, 2).reshape(128, KC * SH)
        )
        in_maps.append({"xb": xb, "wv": wsw})
    return in_maps, A


def epilogue(t_parts, A, inputs):
    f32 = np.float32
    b_g1 = np.asarray(inputs["b_g1"], f32)
    W_g2 = np.asarray(inputs["W_g2"], f32)
    b_g2 = np.asarray(inputs["b_g2"], f32)
    W_head = np.asarray(inputs["W_head"], f32)
    b_head = np.asarray(inputs["b_head"], f32)
    u = np.add.reduce([p.astype(f32) for p in t_parts])  # [14,1] un-aggregated
    t_full = A @ u + np.float32(b_g1 @ W_g2[:, 0])  # conv2 input = x1 @ W_g2
    x2 = A @ t_full + b_g2[0]
    y = float(x2[:, 0] @ W_head[:, 0]) + float(b_head[0])
    return np.array([y], dtype=f32)


def run_on_hw(in_maps, trace=False, tmpdir=None):
    from concourse.bass_utils import run_bass_kernel_spmd

    nc = get_nc()
    return run_bass_kernel_spmd(
        nc,
        in_maps,
        core_ids=list(range(NCORES)),
        trace=trace,
        tmpdir=tmpdir,
    )


def kernel(**inputs):
    in_maps, A = build_host_inputs(inputs)
    res = run_on_hw(in_maps, trace=bool(int(os.environ.get("KERNEL_TRACE", "0"))))
    _CACHE["last_result"] = res
    t_parts = [r["t"] for r in res.results]
    return epilogue(t_parts, A, inputs)
